# revision 34
# baseline (speedup 1.0000x reference)
"""Chamfer loss kernel for Trainium2 (8 NeuronCores, SPMD).

Math: loss = 10 * 0.5 * (mean(sqrt(dist1)) + mean(sqrt(dist2)))
  dist1[j] = min_i ||target_j - state_x_i||^2   (over all state_x)
  dist2[i] = min_j ||state_x_i - target_j||^2   (over all target)

Device strategy (per core k of 8):
  - i (state_x) is sharded: core k owns rows [2048k, 2048(k+1)).
  - j (target) is replicated (stationary matmul operand).
  - Augmented vectors  bhat_j = [1, |b|^2, -2bx, -2by, -2bz],
    ahat_i = [|a|^2, 1, ax, ay, az]  give  bhat_j . ahat_i = d(i, j);
    each side is fp16 hi/lo split (K = 15) so the PE computes the full
    squared-distance tile at near-f32 accuracy.
  - Per group c (128 target points): PE computes the d tile [128 j, 2048 i]
    into PSUM (4 banks, 4x N=512 matmuls).
  - ScalarE (ACT) drains PSUM: relu-clamp + fp16 downcast into SBUF
    (batches of 8 groups in one [128, 8, 2048] tile).
  - VectorE (DVE, the min monopoly) runs two fp16 2x-mode tensor_tensor
    min trees per batch: one folding the batch dim (-> running per-i
    accumulator racc, partition-residue form) and one folding the free
    dim to width 64 + a 1x tensor_reduce (-> exact per-j column mins).
  - Host: partition-residue min, cross-core combine, sqrt/mean epilogue.
"""

import os

import numpy as np

N = 16384
N_CORES = 8
I_PER_CORE = N // N_CORES  # 2048 streaming points per core
JC = 128                   # stationary chunk (output partitions per group)
GROUPS = N // JC           # 128 groups per core
FREE = I_PER_CORE          # 2048 free-dim elements per group
MM_N = 512                 # one PSUM bank of f32 output per matmul
K = 5                      # augmented coordinate count
# fp16 hi/lo split: d = a_hi.b_hi + a_lo.b_hi + a_hi.b_lo (error ~2^-21)
KSPLIT = 3 * K             # contraction dim of the fp16 matmul

# Matmul input dtype: "f16" (hi/lo split, ~2^-21 accurate) or "bf16"
# (hi/lo split, ~2^-15 accurate) — bf16 may stream 2x faster on the PE.
MM_DTYPE = os.environ.get("CHAMFER_MM_DTYPE", "f16")

_CACHE = {}

# Results of the last hardware run (BassKernelResults); test harness reads
# this for exec_time_ns when BASS_TRACE=1.
LAST_RESULTS = None


def _build_nc():
    import concourse.mybir as mybir
    from concourse import bacc
    from concourse.tile import TileContext

    f32 = mybir.dt.float32
    f16 = mybir.dt.float16
    mmdt = f16 if MM_DTYPE == "f16" else mybir.dt.bfloat16
    Op = mybir.AluOpType

    nc = bacc.Bacc(
        "TRN2",
        target_bir_lowering=False,
        debug=False,
        enable_asserts=True,
        num_devices=N_CORES,
    )

    # One input tensor (single DMA → single wait sem on the first matmul):
    # [:, :I_PER_CORE] = streaming ahat slice, [:, I_PER_CORE:] = full bhat.
    ab_aug = nc.dram_tensor(
        "ab_aug", [KSPLIT, I_PER_CORE + N], mmdt, kind="ExternalInput"
    )
    colmin_d = nc.dram_tensor("colmin", [JC, GROUPS], f32, kind="ExternalOutput")
    rowacc_d = nc.dram_tensor("rowacc", [JC, FREE], f16, kind="ExternalOutput")

    QB = 8  # groups per DVE batch (amortizes DVE op init/tail overhead)

    with TileContext(nc) as tc:
        with (
            tc.tile_pool(name="const", bufs=1) as const_pool,
            tc.tile_pool(name="copies", bufs=3) as copy_pool,
            tc.tile_pool(name="tree", bufs=1) as tree_pool,
            tc.tile_pool(name="psum", bufs=2, space="PSUM") as psum_pool,
        ):
            a_rep = const_pool.tile([KSPLIT, I_PER_CORE], mmdt)
            b_rep = const_pool.tile([KSPLIT, N], mmdt)
            # Split input DMAs so group 0's first matmul waits only for a
            # small head slice of each operand.
            nc.sync.dma_start(a_rep[:, :MM_N], ab_aug[:, :MM_N])
            nc.sync.dma_start(a_rep[:, MM_N:], ab_aug[:, MM_N:I_PER_CORE])
            nc.sync.dma_start(
                b_rep[:, :JC], ab_aug[:, I_PER_CORE : I_PER_CORE + JC]
            )
            nc.sync.dma_start(
                b_rep[:, JC : 8 * JC],
                ab_aug[:, I_PER_CORE + JC : I_PER_CORE + 8 * JC],
            )
            nc.sync.dma_start(
                b_rep[:, 8 * JC :], ab_aug[:, I_PER_CORE + 8 * JC :]
            )

            colmin_sb = const_pool.tile([JC, GROUPS], f32)

            racc = [
                const_pool.tile([JC, FREE], f16, name=f"racc{i}") for i in range(2)
            ]

            # Ramp-up batch sizes: DVE work starts after one group instead
            # of a full batch of 8.
            batches = [1, 1, 2, 4, 8, 8] + [QB] * ((GROUPS - 24) // QB)
            assert sum(batches) == GROUPS
            off = 0  # first group of this batch
            for bi, nb in enumerate(batches):
                dcq = copy_pool.tile([JC, QB, FREE], f16, tag="dcq")
                for g in range(nb):
                    c = off + g
                    pt = psum_pool.tile([JC, FREE], f32, tag="pt")
                    for s in range(FREE // MM_N):
                        nc.tensor.matmul(
                            pt[:, s * MM_N : (s + 1) * MM_N],
                            b_rep[:, c * JC : (c + 1) * JC],
                            a_rep[:, s * MM_N : (s + 1) * MM_N],
                            start=True,
                            stop=True,
                        )
                    # ACT drains PSUM: clamp to >=0 + fp16 downcast.
                    nc.scalar.activation(
                        dcq[:, g, :], pt[:], mybir.ActivationFunctionType.Relu
                    )
                # DVE row-accumulate: fold the batch's group slices pairwise
                # (all fp16 2x), then one chain update into racc.
                src = dcq[:, :nb, :]
                m = nb
                while m > 1:
                    t = tree_pool.tile([JC, m // 2, FREE], f16, tag=f"r{m}")
                    nc.vector.tensor_tensor(
                        t[:], src[:, 0 : m // 2, :], src[:, m // 2 : m, :], Op.min
                    )
                    src = t[:]
                    m //= 2
                # src is [JC, 1, FREE] (or the dcq slice when nb == 1)
                if bi == 0:
                    nc.vector.tensor_copy(racc[1][:], dcq[:, 0, :])
                else:
                    nc.vector.tensor_tensor(
                        racc[(bi + 1) % 2][:],
                        racc[bi % 2][:],
                        src[:, 0, :],
                        Op.min,
                    )
                # DVE column-min: batched fp16 2x tree to width 64 + reduce.
                w = FREE // 2
                src = dcq[:, :nb, :]
                while w >= 64:
                    t = tree_pool.tile([JC, QB, w], f16, tag=f"t{w}")
                    nc.vector.tensor_tensor(
                        t[:, :nb, :], src[:, :, :w], src[:, :, w:], Op.min
                    )
                    src = t[:, :nb, :]
                    w //= 2
                nc.vector.tensor_reduce(
                    out=colmin_sb[:, off : off + nb],
                    in_=src,
                    axis=mybir.AxisListType.X,
                    op=Op.min,
                )
                off += nb

            nc.sync.dma_start(colmin_d[:], colmin_sb[:])
            nc.sync.dma_start(rowacc_d[:], racc[len(batches) % 2][:])

    nc.compile()
    return nc


def _augment(pts):
    """pts [N, 3] f32 -> (ahat15 [15, N], bhat15 [15, N]) fp16 hi/lo split.

    ahat = [|a|^2, 1, ax, ay, az]; bhat = [1, |b|^2, -2bx, -2by, -2bz]
    so ahat.bhat = ||a - b||^2.  fp16 split (per column vector v):
    v = v_hi + v_lo + O(2^-22 |v|).  The K=15 layouts
        ahat15 = [a_hi; a_lo; a_hi],  bhat15 = [b_hi; b_hi; b_lo]
    give a_hi.b_hi + a_lo.b_hi + a_hi.b_lo = a.b - a_lo.b_lo - eps.
    """
    pts = np.asarray(pts, dtype=np.float32)
    sq = np.sum(pts * pts, axis=1, dtype=np.float32)
    n = pts.shape[0]
    ahat = np.empty((K, n), dtype=np.float32)
    ahat[0] = sq
    ahat[1] = 1.0
    ahat[2:5] = pts.T
    bhat = np.empty((K, n), dtype=np.float32)
    bhat[0] = 1.0
    bhat[1] = sq
    bhat[2:5] = -2.0 * pts.T

    if MM_DTYPE == "f16":
        dt = np.float16
    else:
        import ml_dtypes

        dt = ml_dtypes.bfloat16
    a_hi = ahat.astype(dt)
    a_lo = (ahat - a_hi.astype(np.float32)).astype(dt)
    b_hi = bhat.astype(dt)
    b_lo = (bhat - b_hi.astype(np.float32)).astype(dt)
    ahat15 = np.concatenate([a_hi, a_lo, a_hi], axis=0)
    bhat15 = np.concatenate([b_hi, b_hi, b_lo], axis=0)
    return ahat15, bhat15


def kernel(state_x, target):
    global LAST_RESULTS
    from concourse.bass_utils import run_bass_kernel_spmd

    state_x = np.asarray(state_x, dtype=np.float32)
    target = np.asarray(target, dtype=np.float32)

    if "nc" not in _CACHE:
        _CACHE["nc"] = _build_nc()
    nc = _CACHE["nc"]

    ahat, _ = _augment(state_x)   # streaming side: state_x
    _, bhat = _augment(target)    # stationary side: target

    in_maps = []
    for k in range(N_CORES):
        sl = slice(k * I_PER_CORE, (k + 1) * I_PER_CORE)
        ab = np.concatenate([ahat[:, sl], bhat], axis=1)
        in_maps.append({"ab_aug": np.ascontiguousarray(ab)})

    res = run_bass_kernel_spmd(nc, in_maps, core_ids=list(range(N_CORES)))
    LAST_RESULTS = res

    # dist2[i] = min_j d(i, j): partition-residue min of the row accumulator.
    dist2 = np.empty(N, dtype=np.float32)
    # dist1[j] = min_i d(i, j): combine per-core partials.
    dist1 = np.full(N, np.inf, dtype=np.float32)
    for k in range(N_CORES):
        out = res.results[k]
        racc = out["rowacc"].astype(np.float32)       # [128, 2048]
        dist2[k * I_PER_CORE : (k + 1) * I_PER_CORE] = racc.min(axis=0)
        colmin = out["colmin"]                        # [128, 128] [p, c]
        dist1 = np.minimum(dist1, colmin.T.reshape(N))

    dist1 = np.maximum(dist1, 0.0)
    dist2 = np.maximum(dist2, 0.0)
    loss = 0.5 * (np.mean(np.sqrt(dist1), dtype=np.float32)
                  + np.mean(np.sqrt(dist2), dtype=np.float32)) * 10.0
    return np.float32(loss)
